# revision 28
# baseline (speedup 1.0000x reference)
import numpy as np
import ml_dtypes
from contextlib import ExitStack

import concourse.bass as bass
import concourse.tile as tile
from concourse import bacc, mybir
from concourse.bass import ts, ds, broadcast_tensor_aps
from concourse.bass_utils import run_bass_kernel_spmd

B, S, D = 1, 2048, 1024
DL, H, DH = 256, 16, 64
E, K, HID = 8, 2, 512
SHID = 512
EPS = 1e-6
NC = 8
TPC = S // NC
HPC = H // NC
NT = S // 128
ND = D // 128
NDL = DL // 128
NH = HID // 128

bf16 = mybir.dt.bfloat16
f32 = mybir.dt.float32
FT = mybir.ActivationFunctionType
ALU = mybir.AluOpType
AX = mybir.AxisListType

_CACHE = {}
DEBUG = False


def _w_sb(nc, pool, dram_ap, rows, cols, dtype, tag):
    t = pool.tile([128, rows // 128, cols], dtype, tag=tag)
    nc.sync.dma_start(t[:], dram_ap.rearrange("(j p) m -> p j m", p=128))
    return t


def build():
    nc = bacc.Bacc("TRN2", target_bir_lowering=False, debug=False,
                   num_devices=NC)

    x_full = nc.dram_tensor("x_full", [S, D], f32, kind="ExternalInput")
    x_own = nc.dram_tensor("x_own", [TPC, D], f32, kind="ExternalInput")
    wq = nc.dram_tensor("wq", [D, 128], bf16, kind="ExternalInput")
    wdkv = nc.dram_tensor("wdkv", [D, DL], bf16, kind="ExternalInput")
    wuk = nc.dram_tensor("wuk", [DL, 128], bf16, kind="ExternalInput")
    wuv = nc.dram_tensor("wuv", [DL, 128], bf16, kind="ExternalInput")
    wo = nc.dram_tensor("wo", [D, D], bf16, kind="ExternalInput")
    wr = nc.dram_tensor("wr", [D, E], f32, kind="ExternalInput")
    shw1 = nc.dram_tensor("shw1", [D, SHID], bf16, kind="ExternalInput")
    shw3 = nc.dram_tensor("shw3", [D, SHID], bf16, kind="ExternalInput")
    shw2 = nc.dram_tensor("shw2", [SHID, D], bf16, kind="ExternalInput")
    rtw1 = nc.dram_tensor("rtw1", [D, HID], bf16, kind="ExternalInput")
    rtw3 = nc.dram_tensor("rtw3", [D, HID], bf16, kind="ExternalInput")
    rtw2 = nc.dram_tensor("rtw2", [HID, D], bf16, kind="ExternalInput")
    onehot = nc.dram_tensor("onehot", [128, E], f32, kind="ExternalInput")
    maskA = nc.dram_tensor("maskA", [128, 256], bf16, kind="ExternalInput")
    maskB = nc.dram_tensor("maskB", [128, 256], bf16, kind="ExternalInput")
    ident = nc.dram_tensor("ident", [128, 128], f32, kind="ExternalInput")

    x_out = nc.dram_tensor("x_out", [TPC, D], f32, kind="ExternalOutput")
    k_out = nc.dram_tensor("k_out", [S, 128], f32, kind="ExternalOutput")
    v_out = nc.dram_tensor("v_out", [S, 128], f32, kind="ExternalOutput")
    aux_out = nc.dram_tensor("aux_out", [1, 1], f32, kind="ExternalOutput")
    dbg = {}
    if DEBUG:
        dbg["d_attn"] = nc.dram_tensor("d_attn", [128, S], f32, kind="ExternalOutput")
        dbg["d_xm"] = nc.dram_tensor("d_xm", [TPC, D], f32, kind="ExternalOutput")
        dbg["d_z"] = nc.dram_tensor("d_z", [S, E], f32, kind="ExternalOutput")
        dbg["d_gate"] = nc.dram_tensor("d_gate", [128, NT], f32, kind="ExternalOutput")
        dbg["d_rs"] = nc.dram_tensor("d_rs", [S, D], bf16, kind="ExternalOutput")
        dbg["d_s"] = nc.dram_tensor("d_s", [128, 256], f32, kind="ExternalOutput")
        dbg["d_p"] = nc.dram_tensor("d_p", [128, 256], bf16, kind="ExternalOutput")
        dbg["d_av"] = nc.dram_tensor("d_av", [128, 256], f32, kind="ExternalOutput")
        dbg["d_rbc"] = nc.dram_tensor("d_rbc", [64, 256], f32, kind="ExternalOutput")

    with tile.TileContext(nc) as tc:
        with (
            tc.tile_pool(name="pmid", bufs=1) as pmid,
            tc.tile_pool(name="px", bufs=2) as px,
            tc.tile_pool(name="psc", bufs=2) as psc,
            tc.tile_pool(name="pp", bufs=2, space="PSUM") as pp,
            tc.tile_pool(name="pps", bufs=2, space="PSUM") as pps,
            tc.tile_pool(name="ppv", bufs=2, space="PSUM") as ppv,
            tc.tile_pool(name="dram", bufs=1, space="DRAM") as dram,
        ):
            build_body(nc, tc, pmid, px, psc, pp, pps, ppv, dram, locals())
    nc.compile()
    return nc


def build_body(nc, tc, pmid, px, psc, pp, pps, ppv, dram, io):
    x_full, x_own = io["x_full"], io["x_own"]
    wq, wdkv, wuk, wuv, wo, wr = (io[k] for k in
                                  ("wq", "wdkv", "wuk", "wuv", "wo", "wr"))
    shw1, shw3, shw2 = io["shw1"], io["shw3"], io["shw2"]
    rtw1, rtw3, rtw2 = io["rtw1"], io["rtw3"], io["rtw2"]
    onehot, maskA, maskB, ident = (io[k] for k in
                                   ("onehot", "maskA", "maskB", "ident"))
    x_out, k_out, v_out, aux_out = (io[k] for k in
                                    ("x_out", "k_out", "v_out", "aux_out"))

    eps_sb = pmid.tile([128, 1], f32, tag="eps")
    nc.vector.memset(eps_sb[:], EPS)

    stA = ExitStack()
    pA = stA.enter_context(tc.tile_pool(name="pA", bufs=1))
    wq_sb = _w_sb(nc, pA, wq[:], D, 128, bf16, "wq")
    wdkv_sb = _w_sb(nc, pA, wdkv[:], D, DL, bf16, "wdkv")
    wuk_sb = _w_sb(nc, pA, wuk[:], DL, 128, bf16, "wuk")
    wuv_sb = _w_sb(nc, pA, wuv[:], DL, 128, bf16, "wuv")
    wo_sb = _w_sb(nc, pA, wo[:], D, D, bf16, "wo")
    wr_sb = _w_sb(nc, pA, wr[:], D, E, f32, "wr")
    mA = pA.tile([128, 256], bf16, tag="mA")
    nc.sync.dma_start(mA[:], maskA[:])
    mB = pA.tile([128, 256], bf16, tag="mB")
    nc.sync.dma_start(mB[:], maskB[:])
    id_sb = pA.tile([128, 128], f32, tag="ident")
    nc.sync.dma_start(id_sb[:], ident[:])

    ag_attn_in = dram.tile([128, S], bf16)
    ag_attn_out = dram.tile([D, S], bf16, addr_space="Shared")
    ag_xn2_in = dram.tile([TPC, D], bf16)
    ag_xn2_out = dram.tile([S, D], bf16, addr_space="Shared")
    ag_z_in = dram.tile([TPC, E], f32)
    ag_z_out = dram.tile([S, E], f32, addr_space="Shared")
    rs_in = dram.tile([S, D], bf16)
    rs_out = dram.tile([TPC, D], bf16)
    cnt_dram = dram.tile([128, 1], f32)

    rg = [list(range(NC))]

    xnT = pA.tile([128, ND, S], bf16, tag="xnT")
    for tt in range(NT):
        xt = px.tile([128, D], f32, tag="xt")
        nc.sync.dma_start(xt[:], x_full[ts(tt, 128), :])
        sq = px.tile([128, D], f32, tag="sf32")
        ssq = psc.tile([128, 1], f32, tag="ssq")
        nc.vector.scalar_tensor_tensor(sq[:], xt[:], 1.0, xt[:],
                                       ALU.mult, ALU.mult, accum_out=ssq[:])
        rms = psc.tile([128, 1], f32, tag="rms")
        nc.scalar.activation(rms[:], ssq[:], FT.Sqrt, scale=1.0 / D, bias=eps_sb[:])
        rstd = psc.tile([128, 1], f32, tag="rstd")
        nc.vector.reciprocal(rstd[:], rms[:])
        xnb = px.tile([128, D], bf16, tag="xnb")
        nc.scalar.activation(xnb[:], xt[:], FT.Copy, scale=rstd[:])
        nc.sync.dma_start_transpose(xnT[:, :, ts(tt, 128)], xnb[:])

    ckvT = pA.tile([128, NDL, S], bf16, tag="ckvT")
    for dl in range(NDL):
        for blk in range(4):
            ps = pp.tile([128, 512], f32, tag="gemm")
            for d in range(ND):
                nc.tensor.matmul(ps[:], wdkv_sb[:, d, ts(dl, 128)],
                                 xnT[:, d, ts(blk, 512)],
                                 start=(d == 0), stop=(d == ND - 1))
            nc.scalar.activation(ckvT[:, dl, ts(blk, 512)], ps[:], FT.Copy)

    qT = pA.tile([128, S], bf16, tag="qT")
    for blk in range(4):
        ps = pp.tile([128, 512], f32, tag="gemm")
        for d in range(ND):
            nc.tensor.matmul(ps[:], wq_sb[:, d, :], xnT[:, d, ts(blk, 512)],
                             start=(d == 0), stop=(d == ND - 1))
        nc.scalar.activation(qT[:, ts(blk, 512)], ps[:], FT.Copy)

    kT = pA.tile([128, S], bf16, tag="kT")
    for blk in range(4):
        ps = pp.tile([128, 512], f32, tag="gemm")
        for dl in range(NDL):
            nc.tensor.matmul(ps[:], wuk_sb[:, dl, :], ckvT[:, dl, ts(blk, 512)],
                             start=(dl == 0), stop=(dl == NDL - 1))
        nc.scalar.activation(kT[:, ts(blk, 512)], ps[:], FT.Copy)

    vaug = pA.tile([128, NT, 128], bf16, tag="vaug")
    ones128b = pA.tile([128, 1], bf16, tag="ones128b")
    nc.vector.memset(ones128b[:], 1.0)
    for tt in range(NT):
        psv = pp.tile([128, 128], f32, tag="kv", bufs=1)
        for dl in range(NDL):
            nc.tensor.matmul(psv[:], ckvT[:, dl, ts(tt, 128)], wuv_sb[:, dl, :],
                             start=(dl == 0), stop=(dl == NDL - 1))
        nc.vector.tensor_copy(vaug[:, tt, :], psv[:])
        vf = px.tile([128, 128], f32, tag="vf")
        nc.scalar.activation(vf[:], psv[:], FT.Copy)
        nc.sync.dma_start(v_out[ts(tt, 128), :], vf[:])

        psk = pp.tile([128, 128], f32, tag="kv", bufs=1)
        for dl in range(NDL):
            nc.tensor.matmul(psk[:], ckvT[:, dl, ts(tt, 128)], wuk_sb[:, dl, :],
                             start=(dl == 0), stop=(dl == NDL - 1))
        kf = px.tile([128, 128], f32, tag="vf")
        nc.scalar.activation(kf[:], psk[:], FT.Copy)
        nc.sync.dma_start(k_out[ts(tt, 128), :], kf[:])

    attnT = [pA.tile([64, S], bf16, tag=f"attnT{h}", name=f"attnT{h}")
             for h in range(HPC)]
    for h in range(HPC):
        for qb in range(8):
            av = ppv.tile([64, 256], f32, tag="av")
            sums_ps = pps.tile([1, 256], f32, tag="sums", bufs=1, name="sums_ps")
            nkt = 2 * qb + 2
            for kt in range(nkt):
                sps = pps.tile([128, 256], f32, tag="sT")
                nc.tensor.matmul(sps[:], kT[ts(h, 64), ts(kt, 128)],
                                 qT[ts(h, 64), ts(qb, 256)],
                                 start=True, stop=True)
                pbf = psc.tile([128, 256], bf16, tag="pbf")
                nc.scalar.activation(pbf[:], sps[:], FT.Exp, scale=0.125)
                if DEBUG and h == 0 and qb == 0 and kt == 0:
                    dsx = psc.tile([128, 256], f32, tag="rec", name="dsx")
                    nc.vector.tensor_copy(dsx[:], sps[:])
                    nc.sync.dma_start(io["dbg"]["d_s"][:], dsx[:])
                if kt == 2 * qb:
                    nc.vector.tensor_mul(pbf[:], pbf[:], mA[:])
                elif kt == 2 * qb + 1:
                    nc.vector.tensor_mul(pbf[:], pbf[:], mB[:])
                if DEBUG and h == 0 and qb == 0 and kt == 0:
                    nc.sync.dma_start(io["dbg"]["d_p"][:], pbf[:])
                nc.tensor.matmul(av[:], vaug[:, kt, ts(h, 64)], pbf[:],
                                 start=(kt == 0), stop=(kt == nkt - 1),
                                 skip_group_check=True)
                nc.tensor.matmul(sums_ps[:], ones128b[:], pbf[:],
                                 start=(kt == 0), stop=(kt == nkt - 1),
                                 skip_group_check=True)
            if DEBUG and h == 0 and qb == 0:
                dav = psc.tile([128, 256], f32, tag="rec", name="dav")
                nc.vector.tensor_copy(dav[0:64, :], av[:])
                nc.vector.tensor_copy(dav[64:65, :], sums_ps[:])
                nc.sync.dma_start(io["dbg"]["d_av"][0:65, :], dav[0:65, :])
            rec = psc.tile([128, 256], f32, tag="rec")
            nc.vector.reciprocal(rec[0:1, :], sums_ps[0:1, :])
            rbc = psc.tile([64, 256], f32, tag="rbc")
            nc.gpsimd.partition_broadcast(rbc[:], rec[0:1, :])
            if DEBUG and h == 0 and qb == 0:
                nc.sync.dma_start(io["dbg"]["d_rbc"][:], rbc[:])
            nc.vector.tensor_mul(attnT[h][:, ts(qb, 256)],
                                 av[:, :], rbc[:])

    dbg = io["dbg"]
    for h in range(HPC):
        nc.sync.dma_start(ag_attn_in[ts(h, 64), :], attnT[h][:])
    if DEBUG:
        for h in range(HPC):
            datt = px.tile([64, S], f32, tag="datt", name=f"datt{h}")
            nc.vector.tensor_copy(datt[:], attnT[h][:])
            nc.sync.dma_start(dbg["d_attn"][ts(h, 64), :], datt[:])
    nc.gpsimd.collective_compute("AllGather", ALU.bypass, replica_groups=rg,
                                 ins=[ag_attn_in[:]], outs=[ag_attn_out[:]])

    pid = nc.sync.partition_id()
    attn_sb = pA.tile([128, ND, TPC], bf16, tag="attn_sb")
    nc.sync.dma_start(
        attn_sb[:],
        ag_attn_out[:].rearrange("(j p) t -> p j t", p=128)[:, :, ds(pid * TPC, TPC)])

    xm = [pmid.tile([128, D], f32, tag=f"xm{t2}", name=f"xm{t2}")
          for t2 in range(2)]
    xn2T_own = pmid.tile([128, ND, TPC], bf16, tag="xn2T_own")
    for t2 in range(2):
        nc.sync.dma_start(xm[t2][:], x_own[ts(t2, 128), :])
        for cb in range(2):
            ps = pp.tile([128, 512], f32, tag="gemm")
            for j in range(ND):
                nc.tensor.matmul(ps[:], attn_sb[:, j, ts(t2, 128)],
                                 wo_sb[:, j, ts(cb, 512)],
                                 start=(j == 0), stop=(j == ND - 1))
            nc.vector.tensor_add(xm[t2][:, ts(cb, 512)],
                                 xm[t2][:, ts(cb, 512)], ps[:])
        sq = px.tile([128, D], f32, tag="sf32")
        ssq = psc.tile([128, 1], f32, tag="ssq")
        nc.vector.scalar_tensor_tensor(sq[:], xm[t2][:], 1.0, xm[t2][:],
                                       ALU.mult, ALU.mult, accum_out=ssq[:])
        rms = psc.tile([128, 1], f32, tag="rms")
        nc.scalar.activation(rms[:], ssq[:], FT.Sqrt, scale=1.0 / D, bias=eps_sb[:])
        rstd = psc.tile([128, 1], f32, tag="rstd")
        nc.vector.reciprocal(rstd[:], rms[:])
        xn2b = px.tile([128, D], bf16, tag="xnb")
        nc.scalar.activation(xn2b[:], xm[t2][:], FT.Copy, scale=rstd[:])
        nc.sync.dma_start_transpose(xn2T_own[:, :, ts(t2, 128)], xn2b[:])
        nc.sync.dma_start(ag_xn2_in[ts(t2, 128), :], xn2b[:])
        xn2f = px.tile([128, D], f32, tag="sf32")
        nc.scalar.activation(xn2f[:], xm[t2][:], FT.Copy, scale=rstd[:])
        xn2T_f = px.tile([128, ND, 128], f32, tag="xn2T_f")
        for d in range(ND):
            pst = pps.tile([128, 256], f32, tag="sums", bufs=1, name="pst")
            nc.tensor.transpose(pst[:, 0:128], xn2f[:, ts(d, 128)], id_sb[:])
            nc.vector.tensor_copy(xn2T_f[:, d, :], pst[:, 0:128])
        psz = pp.tile([128, E], f32, tag="kv", bufs=1)
        for d in range(ND):
            nc.tensor.matmul(psz[:], xn2T_f[:, d, :], wr_sb[:, d, :],
                             start=(d == 0), stop=(d == ND - 1))
        zf = psc.tile([128, E], f32, tag="zf")
        nc.vector.tensor_copy(zf[:], psz[:])
        nc.sync.dma_start(ag_z_in[ts(t2, 128), :], zf[:])

    nc.gpsimd.collective_compute("AllGather", ALU.bypass, replica_groups=rg,
                                 ins=[ag_xn2_in[:]], outs=[ag_xn2_out[:]])
    nc.gpsimd.collective_compute("AllGather", ALU.bypass, replica_groups=rg,
                                 ins=[ag_z_in[:]], outs=[ag_z_out[:]])
    if DEBUG:
        for t2 in range(2):
            nc.sync.dma_start(dbg["d_xm"][ts(t2, 128), :], xm[t2][:])
    stA.close()

    stB = ExitStack()
    pB = stB.enter_context(tc.tile_pool(name="pB", bufs=1))
    oh_sb = pB.tile([128, E], f32, tag="onehot")
    nc.sync.dma_start(oh_sb[:], onehot[:])
    ones_sb = pB.tile([128, 1], f32, tag="ones")
    nc.vector.memset(ones_sb[:], 1.0)
    z_ce = psc.tile([128, NT, E], f32, tag="z_ce")
    nc.sync.dma_start(z_ce[:], ag_z_out[:].rearrange("(c p) e -> p c e", p=128))
    m1 = psc.tile([128, NT], f32, tag="m1")
    nc.vector.tensor_reduce(m1[:], z_ce[:], AX.X, ALU.max)
    mask1 = psc.tile([128, NT, E], f32, tag="mask1")
    a1, b1 = broadcast_tensor_aps(z_ce[:], m1[:].rearrange("p (c o) -> p c o", o=1))
    nc.vector.tensor_tensor(mask1[:], a1, b1, ALU.is_ge)
    zm = psc.tile([128, NT, E], f32, tag="zm")
    nc.vector.scalar_tensor_tensor(zm[:], mask1[:], -1e30, z_ce[:],
                                   ALU.mult, ALU.add)
    m2 = psc.tile([128, NT], f32, tag="m2")
    nc.vector.tensor_reduce(m2[:], zm[:], AX.X, ALU.max)
    mask2 = psc.tile([128, NT, E], f32, tag="mask2")
    a2, b2 = broadcast_tensor_aps(zm[:], m2[:].rearrange("p (c o) -> p c o", o=1))
    nc.vector.tensor_tensor(mask2[:], a2, b2, ALU.is_ge)
    dgap = psc.tile([128, NT], f32, tag="dgap")
    nc.vector.tensor_sub(dgap[:], m1[:], m2[:])
    g1 = psc.tile([128, NT], f32, tag="g1")
    nc.scalar.activation(g1[:], dgap[:], FT.Sigmoid)
    g2 = psc.tile([128, NT], f32, tag="g2")
    nc.vector.tensor_scalar(g2[:], g1[:], -1.0, 1.0, ALU.mult, ALU.add)
    gfull = psc.tile([128, NT, E], f32, tag="gfull")
    am, bm = broadcast_tensor_aps(mask1[:], g1[:].rearrange("p (c o) -> p c o", o=1))
    nc.vector.tensor_tensor(gfull[:], am, bm, ALU.mult)
    t2m = psc.tile([128, NT, E], f32, tag="t2m")
    am2, bm2 = broadcast_tensor_aps(mask2[:], g2[:].rearrange("p (c o) -> p c o", o=1))
    nc.vector.tensor_tensor(t2m[:], am2, bm2, ALU.mult)
    nc.vector.tensor_add(gfull[:], gfull[:], t2m[:])
    gsel = psc.tile([128, NT, E], f32, tag="gsel")
    ao, bo = broadcast_tensor_aps(gfull[:],
                                  oh_sb[:].rearrange("p (o e) -> p o e", o=1))
    nc.vector.tensor_tensor(gsel[:], ao, bo, ALU.mult)
    gate_e = psc.tile([128, NT], f32, tag="gate_e")
    nc.vector.tensor_reduce(gate_e[:], gsel[:], AX.X, ALU.add)

    if DEBUG:
        nc.sync.dma_start(dbg["d_gate"][:], gate_e[:])
        dz = psc.tile([128, NT, E], f32, tag="z_ce", name="dz")
        nc.vector.tensor_copy(dz[:], z_ce[:])
        nc.sync.dma_start(dbg["d_z"][:].rearrange("(c p) e -> p c e", p=128), dz[:])

    z_ec = psc.tile([128, E, NT], f32, tag="z_ec")
    nc.sync.dma_start(z_ec[:], ag_z_out[:].rearrange("(c p) e -> p e c", p=128))
    exp_ce = psc.tile([128, NT, E], f32, tag="exp_ce")
    nc.scalar.activation(exp_ce[:], z_ce[:], FT.Exp)
    se = psc.tile([128, NT], f32, tag="se")
    nc.vector.tensor_reduce(se[:], exp_ce[:], AX.X, ALU.add)
    rse = psc.tile([128, NT], f32, tag="rse")
    nc.vector.reciprocal(rse[:], se[:])
    exp_ec = psc.tile([128, E, NT], f32, tag="exp_ec")
    nc.scalar.activation(exp_ec[:], z_ec[:], FT.Exp)
    probs_ec = psc.tile([128, E, NT], f32, tag="probs_ec")
    ap1, ap2 = broadcast_tensor_aps(exp_ec[:],
                                    rse[:].rearrange("p (o c) -> p o c", o=1))
    nc.vector.tensor_tensor(probs_ec[:], ap1, ap2, ALU.mult)
    ps_e = psc.tile([128, E], f32, tag="ps_e")
    nc.vector.tensor_reduce(ps_e[:], probs_ec[:], AX.X, ALU.add)
    m12 = psc.tile([128, NT, E], f32, tag="m12")
    nc.vector.tensor_add(m12[:], mask1[:], mask2[:])
    pcnt = pp.tile([128, 1], f32, tag="kv", bufs=1)
    nc.tensor.matmul(pcnt[:], m12[:].rearrange("p c e -> p (c e)"), ones_sb[:],
                     start=True, stop=True)
    cnt_sb = psc.tile([128, 1], f32, tag="cnt_sb")
    nc.vector.tensor_copy(cnt_sb[:], pcnt[:])
    nc.sync.dma_start(cnt_dram[:], cnt_sb[:])
    cnt_e = psc.tile([E, NT], f32, tag="cnt_e")
    nc.sync.dma_start(cnt_e[:],
                      cnt_dram[:].rearrange("(c e) o -> e (c o)", e=E))
    counts = psc.tile([E, 1], f32, tag="counts")
    nc.vector.tensor_reduce(counts[:], cnt_e[:], AX.X, ALU.add)
    psum_e = pp.tile([E, 1], f32, tag="kv", bufs=1)
    nc.tensor.matmul(psum_e[:], ps_e[:], ones_sb[:], start=True, stop=True)
    auxv = psc.tile([E, 1], f32, tag="auxv")
    nc.vector.tensor_tensor(auxv[:], counts[:], psum_e[:], ALU.mult)
    nc.vector.tensor_scalar_mul(auxv[:], auxv[:],
                                float(E * E / (K * S) / S))
    pax = pp.tile([1, 1], f32, tag="kv", bufs=1)
    nc.tensor.matmul(pax[:], auxv[:], ones_sb[0:E, :], start=True, stop=True)
    aux_sb = psc.tile([1, 1], f32, tag="aux_sb")
    nc.vector.tensor_copy(aux_sb[:], pax[:])
    nc.sync.dma_start(aux_out[:], aux_sb[:])

    rtw1_sb = _w_sb(nc, pB, rtw1[:], D, HID, bf16, "rtw1")
    rtw3_sb = _w_sb(nc, pB, rtw3[:], D, HID, bf16, "rtw3")
    rtw2_sb = _w_sb(nc, pB, rtw2[:], HID, D, bf16, "rtw2")
    xn2T = pB.tile([128, ND, S], bf16, tag="xn2T")
    for tt in range(NT):
        nc.sync.dma_start_transpose(xn2T[:, :, ts(tt, 128)],
                                    ag_xn2_out[ts(tt, 128), :])
    hT = pB.tile([128, NH, S], bf16, tag="hT")
    for ft in range(NH):
        for blk in range(4):
            psa = pp.tile([128, 512], f32, tag="gemm")
            for d in range(ND):
                nc.tensor.matmul(psa[:], rtw1_sb[:, d, ts(ft, 128)],
                                 xn2T[:, d, ts(blk, 512)],
                                 start=(d == 0), stop=(d == ND - 1))
            psb = pp.tile([128, 512], f32, tag="gemm")
            for d in range(ND):
                nc.tensor.matmul(psb[:], rtw3_sb[:, d, ts(ft, 128)],
                                 xn2T[:, d, ts(blk, 512)],
                                 start=(d == 0), stop=(d == ND - 1))
            sil = px.tile([128, 512], bf16, tag="sil")
            nc.scalar.activation(sil[:], psa[:], FT.Silu)
            nc.vector.tensor_mul(hT[:, ft, ts(blk, 512)], sil[:], psb[:])
    for tt in range(NT):
        for cb in range(2):
            ps = pp.tile([128, 512], f32, tag="gemm")
            for ft in range(NH):
                nc.tensor.matmul(ps[:], hT[:, ft, ts(tt, 128)],
                                 rtw2_sb[:, ft, ts(cb, 512)],
                                 start=(ft == 0), stop=(ft == NH - 1))
            yb = px.tile([128, 512], bf16, tag="sil")
            nc.scalar.activation(yb[:], ps[:], FT.Copy, scale=gate_e[:, tt:tt + 1])
            nc.sync.dma_start(rs_in[ts(tt, 128), ts(cb, 512)], yb[:])
            if DEBUG:
                nc.sync.dma_start(dbg["d_rs"][ts(tt, 128), ts(cb, 512)], yb[:])
    nc.gpsimd.collective_compute("ReduceScatter", ALU.add, replica_groups=rg,
                                 ins=[rs_in[:]], outs=[rs_out[:]])

    shw1_sb = _w_sb(nc, pB, shw1[:], D, SHID, bf16, "shw1")
    shw3_sb = _w_sb(nc, pB, shw3[:], D, SHID, bf16, "shw3")
    shw2_sb = _w_sb(nc, pB, shw2[:], SHID, D, bf16, "shw2")
    shT = pB.tile([128, NH, TPC], bf16, tag="shT")
    for ft in range(NH):
        psa = pp.tile([128, 256], f32, tag="gemm")
        for d in range(ND):
            nc.tensor.matmul(psa[:], shw1_sb[:, d, ts(ft, 128)],
                             xn2T_own[:, d, :],
                             start=(d == 0), stop=(d == ND - 1))
        psb = pp.tile([128, 256], f32, tag="gemm")
        for d in range(ND):
            nc.tensor.matmul(psb[:], shw3_sb[:, d, ts(ft, 128)],
                             xn2T_own[:, d, :],
                             start=(d == 0), stop=(d == ND - 1))
        sil = px.tile([128, 256], bf16, tag="sil")
        nc.scalar.activation(sil[:], psa[:], FT.Silu)
        nc.vector.tensor_mul(shT[:, ft, :], sil[:], psb[:])
    for t2 in range(2):
        for cb in range(2):
            ps = pp.tile([128, 512], f32, tag="gemm")
            for ft in range(NH):
                nc.tensor.matmul(ps[:], shT[:, ft, ts(t2, 128)],
                                 shw2_sb[:, ft, ts(cb, 512)],
                                 start=(ft == 0), stop=(ft == NH - 1))
            nc.vector.tensor_add(xm[t2][:, ts(cb, 512)],
                                 xm[t2][:, ts(cb, 512)], ps[:])

    for t2 in range(2):
        rt = px.tile([128, D], bf16, tag="xnb")
        nc.sync.dma_start(rt[:], rs_out[ts(t2, 128), :])
        xo = px.tile([128, D], f32, tag="sf32")
        nc.vector.tensor_add(xo[:], xm[t2][:], rt[:])
        nc.sync.dma_start(x_out[ts(t2, 128), :], xo[:])
    stB.close()



def _prep_inputs(inputs):
    X = np.asarray(inputs["X"], np.float32).reshape(S, D)
    anw = np.asarray(inputs["attn_norm_w"], np.float32)
    fnw = np.asarray(inputs["ffn_norm_w"], np.float32)
    bf = ml_dtypes.bfloat16

    Wq_f = (np.asarray(inputs["Wq"], np.float32) * anw[:, None])
    Wdkv_f = (np.asarray(inputs["Wdkv"], np.float32) * anw[:, None]).astype(bf)
    Wuk = np.asarray(inputs["Wuk"], np.float32)
    Wuv = np.asarray(inputs["Wuv"], np.float32)
    Wo = np.asarray(inputs["Wo"], np.float32).astype(bf)
    Wr_f = (np.asarray(inputs["Wr"], np.float32) * fnw[:, None]).astype(np.float32)
    shw1 = (np.asarray(inputs["sh_w1"], np.float32) * fnw[:, None]).astype(bf)
    shw3 = (np.asarray(inputs["sh_w3"], np.float32) * fnw[:, None]).astype(bf)
    shw2 = np.asarray(inputs["sh_w2"], np.float32).astype(bf)
    rtw1 = (np.asarray(inputs["rt_w1"], np.float32) * fnw[None, :, None]).astype(bf)
    rtw3 = (np.asarray(inputs["rt_w3"], np.float32) * fnw[None, :, None]).astype(bf)
    rtw2 = np.asarray(inputs["rt_w2"], np.float32).astype(bf)

    p = np.arange(128)[:, None]
    f = np.arange(256)[None, :]
    maskA = (p - f <= 0).astype(bf)
    maskB = (128 + p - f <= 0).astype(bf)
    ident = np.eye(128, dtype=np.float32)

    in_maps = []
    for c in range(NC):
        hc = slice(128 * c, 128 * c + 128)
        onehot = np.zeros((128, E), np.float32)
        onehot[:, c] = 1.0
        in_maps.append({
            "x_full": X,
            "x_own": X[TPC * c: TPC * (c + 1)],
            "wq": Wq_f[:, hc].astype(bf),
            "wdkv": Wdkv_f,
            "wuk": Wuk[:, hc].astype(bf),
            "wuv": Wuv[:, hc].astype(bf),
            "wo": Wo,
            "wr": Wr_f,
            "shw1": shw1, "shw3": shw3, "shw2": shw2,
            "rtw1": np.ascontiguousarray(rtw1[c]),
            "rtw3": np.ascontiguousarray(rtw3[c]),
            "rtw2": np.ascontiguousarray(rtw2[c]),
            "onehot": onehot,
            "maskA": maskA, "maskB": maskB,
            "ident": ident,
        })
    return in_maps


def _assemble(results):
    X_out = np.concatenate([results[c]["x_out"] for c in range(NC)], axis=0)
    k = np.concatenate([results[c]["k_out"] for c in range(NC)], axis=1)
    v = np.concatenate([results[c]["v_out"] for c in range(NC)], axis=1)
    aux = np.float32(results[0]["aux_out"][0, 0])
    return (X_out.reshape(B, S, D).astype(np.float32),
            aux,
            k.reshape(B, S, H, DH).astype(np.float32),
            v.reshape(B, S, H, DH).astype(np.float32))


def get_nc():
    if "nc" not in _CACHE:
        _CACHE["nc"] = build()
    return _CACHE["nc"]


def kernel(**inputs):
    nc = get_nc()
    in_maps = _prep_inputs(inputs)
    res = run_bass_kernel_spmd(nc, in_maps, list(range(NC)))
    return _assemble(res.results)
